# revision 1
# baseline (speedup 1.0000x reference)
"""Causal CoreAttention kernel for Trainium2 (Bass/Tile), 8-core SPMD.

Problem: B=2, H=16, S=2048, D=128 fp32 causal attention.
Sharding: B*H=32 heads -> 4 heads per core across 8 cores.

Design (cost-model-driven; ACT exp is the bottleneck engine at ~58us/core
of mandatory columns):
  - S^T layout: per k-strip kt, S^T[kt,q] = K^T.T @ Q^T via bf16 matmuls;
    exp evacuates PSUM->bf16 P^T strips; PV accumulates
    O[qt] = sum_kt P^T[kt,qt].T @ [V|1] with the softmax denominator in
    column 128; DVE reciprocal + tensor_scalar normalizes.
  - Strips are packed into 16 activation instructions per head (<=1536-col
    PSUM tiles, 3 banks x 2 slots; small strips 9..15 share tiles) to
    amortize ACT's ~185ns per-instruction overhead.
  - Q,K transposes: head 0 uses direct fp32 PE transposes (PE idle at
    startup, warmed by dummy transposes); steady-state heads cast to bf16
    on Pool (gpsimd) and use the XBAR DMA transpose straight into SBUF.
  - Input DMAs issue from SP (HWDGE); head-0 loads are chunked and
    interleaved so the first activation starts ~6.5us in.
  - Software pipelining: head h+1's casts/xposes burst at head h's start,
    loads prefetch up to two heads ahead (triple-buffered naturals), PV
    chains drain behind exp with a per-group PE budget, and the last head
    finishes with single-strip groups + per-qt output DMA to minimize the
    tail.
Measured (TimelineSim cost model, matches harness): 91717 ns/core,
rel err 3.37e-03 on hardware (baseline was 112642 ns).
"""
import math

import numpy as np

import concourse.bass as bass
import concourse.mybir as mybir
import concourse.tile as tile
from concourse.bass_utils import run_bass_kernel_spmd
from concourse.masks import make_identity, make_upper_triangular

B, H, S, D = 2, 16, 2048, 128
NCORES = 8
HPC = (B * H) // NCORES          # heads per core
NT = S // 128                    # 16 tiles per head
SCALE = 1.0 / math.sqrt(D)

MAX_WAITS = 1  # walrus TRN2 encodes at most 1 sync-wait per instruction


def _split_waits(nc):
    """Tile emits >1 sem-wait on some instructions; hoist extras onto NoOps
    inserted just before, on the same (in-order) engine."""
    for f in nc.m.functions:
        for bb in f.blocks:
            insts = bb.instructions
            out = []
            changed = False
            for inst in insts:
                si = inst.sync_info
                if si is not None and len(si.on_wait) > MAX_WAITS:
                    waits = list(si.on_wait)
                    extra, keep = waits[:-MAX_WAITS], waits[-MAX_WAITS:]
                    for j in range(0, len(extra), MAX_WAITS):
                        nop = mybir.InstNoOp(
                            name=f"{inst.name}-ws{j}", engine=inst.engine)
                        nop.sync_info = mybir.SyncInfo(
                            on_wait=extra[j:j + MAX_WAITS], on_update=[])
                        out.append(nop)
                    inst.sync_info = mybir.SyncInfo(
                        on_wait=keep, on_update=list(si.on_update))
                    changed = True
                out.append(inst)
            if changed:
                insts[:] = out


def _round128(x):
    return ((x + 127) // 128) * 128


def make_groups():
    """Activation groups: list of lists of pieces (kt, a, b) with
    sum of lens <= 1536 and pieces consecutive in ptall order."""
    groups = []
    for kt in range(9):
        L = S - 128 * kt
        if L > 1536:
            half = _round128((L + 1) // 2)
            groups.append([(kt, 0, half)])
            groups.append([(kt, half, L)])
        else:
            groups.append([(kt, 0, L)])
    groups.append([(9, 0, 896)])
    groups.append([(10, 0, 768), (11, 0, 640)])
    groups.append([(12, 0, 512), (13, 0, 384), (14, 0, 256), (15, 0, 128)])
    return groups


GROUPS = make_groups()
# last head: final strips as singles so the PV tail drains early
GROUPS_LAST = GROUPS[:-1] + [[(kt, 0, S - 128 * kt)] for kt in range(12, 16)]
# head 0: small leading pieces ordered by q-chunk arrival so the first
# activations start while the DMA is still in flight
GROUPS_H0 = ([[(0, 0, 512)], [(0, 512, 1024)], [(0, 1024, 1536)],
              [(1, 0, 1024)], [(0, 1536, 2048)], [(1, 1024, 1920)],
              [(2, 0, 896)], [(2, 896, 1792)], [(3, 0, 896)],
              [(3, 896, 1664)]] + GROUPS[8:])


def strip_done_at(groups):
    d = {}
    for gi, grp in enumerate(groups):
        for (kt, a, b) in grp:
            if b == S - 128 * kt:
                d[kt] = gi
    return d


def build_nc2(pv_cap=2, prep_cap=2, qk_slot=1536, pv_budget=7, pv_budget_last=7, GI_PREFETCH=6, xbar=True, DVE_EXP_KTS=(), out_pool=False):
    fp32 = mybir.dt.float32
    bf16 = mybir.dt.bfloat16

    nc = bass.Bass("TRN2", target_bir_lowering=False)
    q = nc.dram_tensor("q", [HPC, S, D], fp32, kind="ExternalInput").ap()
    k = nc.dram_tensor("k", [HPC, S, D], fp32, kind="ExternalInput").ap()
    v = nc.dram_tensor("v", [HPC, S, D], fp32, kind="ExternalInput").ap()
    o = nc.dram_tensor("o", [HPC, S, D], fp32, kind="ExternalOutput").ap()

    # ptall strip offsets (packed causal layout)
    off = []
    t = 0
    for kt in range(NT):
        off.append(t)
        t += S - 128 * kt
    pt_len = t  # 17408

    dram = {"q": q, "k": k, "v": v}

    with tile.TileContext(nc) as tc:
        with tc.tile_pool(name="const", bufs=1) as constp, \
             tc.tile_pool(name="nat", bufs=2) as natp, \
             tc.tile_pool(name="b16", bufs=2) as b16p, \
             tc.tile_pool(name="qkT", bufs=2) as qktp, \
             tc.tile_pool(name="vaug", bufs=2) as vaugp, \
             tc.tile_pool(name="pt", bufs=2) as ptp, \
             tc.tile_pool(name="osb", bufs=2) as osbp, \
             tc.tile_pool(name="rc", bufs=2) as rcp, \
             tc.tile_pool(name="schr", bufs=2) as schrp, \
             tc.tile_pool(name="qk_ps", bufs=2, space="PSUM") as qkps, \
             tc.tile_pool(name="pv_ps", bufs=2, space="PSUM") as pvps:

            identf = constp.tile([128, 128], fp32, tag="identf")
            make_identity(nc, identf[:])
            identb = constp.tile([128, 128], bf16, tag="identb")
            make_identity(nc, identb[:])
            ltri = constp.tile([128, 128], bf16, tag="ltri")
            # keep P^T[kk, qq] where kk <= qq (partition <= free)
            make_upper_triangular(nc, ltri[:], val=1.0, diag=True)

            # PE p-state warm-up: dummy transposes with no consumers so the
            # first real transposes run at full clock
            for wi in range(6):
                wps = pvps.tile([128, 128], fp32, tag="pv", name=f"warm{wi}")
                nc.tensor.transpose(wps[:], identf[:], identf[:])

            # per-head tiles, created lazily
            tiles = {}

            def head_tiles(h):
                if h in tiles:
                    return tiles[h]
                d = {
                    "qn": natp.tile([128, NT, 128], fp32, tag="qn",
                                    name=f"qn{h}", bufs=3),
                    "kn": natp.tile([128, NT, 128], fp32, tag="kn",
                                    name=f"kn{h}", bufs=3),
                    "vn": natp.tile([128, NT, 128], fp32, tag="vn",
                                    name=f"vn{h}"),
                    "qT": qktp.tile([128, S], bf16, tag="qT", name=f"qT{h}"),
                    "kT": qktp.tile([128, S], bf16, tag="kT", name=f"kT{h}"),
                    "va": vaugp.tile([128, NT, 130], bf16, tag="va",
                                     name=f"va{h}"),
                    "pt": ptp.tile([128, pt_len], bf16, tag="pt",
                                   name=f"pt{h}"),
                    "osb": osbp.tile([128, NT, 128], fp32, tag="osb",
                                     name=f"osb{h}"),
                    "rc": rcp.tile([128, NT], fp32, tag="rc", name=f"rc{h}"),
                }
                tiles[h] = d
                return d

            def emit_load(h, name, chunked, g=None, eng=None):
                dst = head_tiles(h)[name[0] + "n"]
                ap = dram[name][h].rearrange("(t p) d -> p t d", p=128)
                eng = eng or nc.sync
                if chunked:
                    eng.dma_start(dst[:, 4 * g:4 * g + 4, :],
                                  ap[:, 4 * g:4 * g + 4, :])
                else:
                    eng.dma_start(dst[:], ap)

            def h0_vcast_item(g):
                return lambda: prep_vcast(0, g)

            def prep_cast(h, which, g):
                d = head_tiles(h)
                src = d[which + "n"]
                if h == 0:
                    # startup: PE-transpose the fp32 naturals directly (PE is
                    # idle here; no Pool cast on the critical path), DVE evac
                    # converts to bf16
                    pst = pvps.tile([128, 512], fp32, tag="pv",
                                    name=f"tr{which}{h}_{g}")
                    for j in range(4):
                        nc.tensor.transpose(
                            pst[:, 128 * j:128 * (j + 1)],
                            src[:, 4 * g + j, :], identf[:])
                    nc.vector.tensor_copy(
                        d[which + "T"][:, 512 * g:512 * (g + 1)], pst[:])
                else:
                    # steady state: Pool bf16 cast + transpose
                    if which + "b" not in d:
                        d[which + "b"] = b16p.tile(
                            [128, NT, 128], bf16, tag=which + "b",
                            name=f"{which}b{h}")
                    dst = d[which + "b"]
                    nc.gpsimd.tensor_copy(dst[:, 4 * g:4 * g + 4, :],
                                          src[:, 4 * g:4 * g + 4, :])
                    if xbar:
                        nc.sync.dma_start_transpose(
                            d[which + "T"][:, 512 * g:512 * (g + 1)]
                            .rearrange("d (t p) -> d t p", p=128),
                            dst[:, 4 * g:4 * g + 4, :])
                    else:
                        pst = pvps.tile([128, 512], bf16, tag="pv",
                                        name=f"tr{which}{h}_{g}")
                        for j in range(4):
                            nc.tensor.transpose(
                                pst[:, 128 * j:128 * (j + 1)],
                                dst[:, 4 * g + j, :], identb[:])
                        nc.vector.tensor_copy(
                            d[which + "T"][:, 512 * g:512 * (g + 1)], pst[:])

            def prep_vcast(h, g):
                d = head_tiles(h)
                nc.gpsimd.tensor_copy(
                    d["va"][:, 4 * g:4 * g + 4, 0:128],
                    d["vn"][:, 4 * g:4 * g + 4, :])

            def prep_ones(h):
                nc.gpsimd.memset(head_tiles(h)["va"][:, :, 128:129], 1.0)

            def OUT_ENG():
                return nc.gpsimd if out_pool else nc.sync

            # Schraudolph fast-exp constants (DVE offload path):
            # exp(s*SCALE) ~ bitcast(int32(s*SA + SB)); SB centers the
            # mantissa-linear error to about +/-4.2 percent
            SA = SCALE * 1.4426950408889634 * float(1 << 23)
            SB = (127.0 - 0.043) * float(1 << 23)

            def emit_group(h, grp, dve_exp=False):
                d = head_tiles(h)
                glen = sum(b - a for (_, a, b) in grp)
                ps = qkps.tile([128, qk_slot], fp32, tag="qk",
                               name=f"qk{h}_{grp[0][0]}_{grp[0][1]}")
                c = 0
                for (kt, a, b) in grp:
                    q0 = 128 * kt
                    pos = c
                    pend = c + (b - a)
                    while pos < pend:
                        nxt = min(pend, (pos // 512 + 1) * 512)
                        nc.tensor.matmul(
                            ps[:, pos:nxt],
                            d["kT"][:, q0:q0 + 128],
                            d["qT"][:, q0 + a + (pos - c):q0 + a + (nxt - c)],
                            start=True, stop=True)
                        pos = nxt
                    c = pend
                kt0, a0, _ = grp[0]
                ptout = d["pt"][:, off[kt0] + a0:off[kt0] + a0 + glen]
                if dve_exp:
                    si = schrp.tile([128, 1536], mybir.dt.int32, tag="si",
                                    name=f"si{h}_{kt0}")
                    nc.vector.tensor_scalar(
                        out=si[:, 0:glen], in0=ps[:, 0:glen],
                        scalar1=SA, scalar2=SB,
                        op0=mybir.AluOpType.mult, op1=mybir.AluOpType.add)
                    nc.gpsimd.tensor_copy(
                        ptout, si[:, 0:glen].bitcast(mybir.dt.float32))
                else:
                    nc.scalar.activation(
                        ptout, ps[:, 0:glen],
                        mybir.ActivationFunctionType.Exp, scale=SCALE)
                # diag masks for strips whose first piece is in this group
                for (kt, a, b) in grp:
                    if a == 0:
                        nc.vector.tensor_mul(
                            d["pt"][:, off[kt]:off[kt] + 128],
                            d["pt"][:, off[kt]:off[kt] + 128],
                            ltri[:])

            def emit_pv_mm(h, qt, po, kt0, kt1):
                d = head_tiles(h)
                for kt in range(kt0, kt1 + 1):
                    nc.tensor.matmul(
                        po[:, 0:129],
                        d["pt"][:, off[kt] + (qt - kt) * 128:
                                off[kt] + (qt - kt) * 128 + 128],
                        d["va"][:, kt, 0:129],
                        start=(kt == 0), stop=(kt == qt))

            def emit_pv_norm(h, qt, po):
                d = head_tiles(h)
                nc.vector.reciprocal(d["rc"][:, qt:qt + 1], po[:, 128:129])
                nc.vector.tensor_scalar_mul(
                    d["osb"][:, qt, :], po[:, 0:128], d["rc"][:, qt:qt + 1])
                oap = o[h].rearrange("(t p) d -> p t d", p=128)
                if h == HPC - 1 and qt >= 12:
                    # tail: per-qt DMA so the final transfer is small
                    nc.sync.dma_start(oap[:, qt:qt + 1, :],
                                      d["osb"][:, qt:qt + 1, :])
                elif qt % 4 == 3:
                    g = qt // 4
                    OUT_ENG().dma_start(oap[:, 4 * g:4 * g + 4, :],
                                        d["osb"][:, 4 * g:4 * g + 4, :])

            def emit_pv(h, qt):
                po = pvps.tile([128, 129], fp32, tag="pv",
                               name=f"pv{h}_{qt}")
                emit_pv_mm(h, qt, po, 0, qt)
                emit_pv_norm(h, qt, po)

            # ---------------- emission schedule ----------------
            prep_q = []      # closures
            pv_q = []        # (h, qt) eligible, in order
            pv_next = {}     # h -> next qt to make eligible
            partials = {}    # last head qt -> open PSUM accumulator


            # head 0: chunked loads (q-first interleave); v chunks after q
            # so Pool's vcasts run before the h1 cast burst
            emit_load(0, "q", True, 0)
            emit_load(0, "k", True, 0)
            emit_load(0, "q", True, 1)
            emit_load(0, "q", True, 2)
            emit_load(0, "q", True, 3)
            emit_load(0, "v", True, 0)
            emit_load(0, "v", True, 1)
            emit_load(0, "k", True, 1)
            emit_load(0, "v", True, 2)
            emit_load(0, "v", True, 3)
            emit_load(0, "k", True, 2)
            emit_load(0, "k", True, 3)
            prep_cast(0, "q", 0)
            prep_cast(0, "k", 0)
            prep_cast(0, "q", 1)
            for g in range(4):
                prep_vcast(0, g)
            prep_ones(0)
            prep_q.extend([
                lambda: prep_cast(0, "q", 2),
                lambda: prep_cast(0, "q", 3),
                lambda: prep_cast(0, "k", 1),
                lambda: prep_cast(0, "k", 2),
                lambda: prep_cast(0, "k", 3),
            ])

            for h in range(HPC):
                pv_next[h] = 0
                if h == 0:
                    groups = GROUPS_H0
                elif h == HPC - 1:
                    groups = GROUPS_LAST
                else:
                    groups = GROUPS
                done_at = strip_done_at(groups)
                if h + 1 < HPC:
                    # loads for h+1: at h0 issue here; later heads were
                    # prefetched mid-previous-head
                    if h == 0:
                        emit_load(1, "k", False)
                        emit_load(1, "q", False)
                        emit_load(1, "v", False)
                    prep_cast(h + 1, "k", 0)
                    for g in range(4):
                        prep_cast(h + 1, "q", g)
                    for g in range(1, 4):
                        prep_cast(h + 1, "k", g)
                    for g in range(4):
                        prep_vcast(h + 1, g)
                    prep_ones(h + 1)
                for gi, grp in enumerate(groups):
                    emit_group(h, grp, dve_exp=grp[0][0] in DVE_EXP_KTS)
                    if gi == GI_PREFETCH and h + 2 < HPC:
                        # prefetch head h+2 loads mid-head so the next head's
                        # cast burst starts with data already resident
                        emit_load(h + 2, "k", False)
                        emit_load(h + 2, "q", False)
                        emit_load(h + 2, "v", False)
                    # eligibility: strips done as of previous group
                    # (lag 0 in the last head's tail groups)
                    lag0 = h == HPC - 1 and gi >= len(groups) - 4
                    if gi >= 1 or lag0:
                        done_gi = gi if lag0 else gi - 1
                        while (pv_next[h] < NT
                               and done_at[pv_next[h]] <= done_gi):
                            pv_q.append((h, pv_next[h]))
                            pv_next[h] += 1

                    for _ in range(prep_cap):
                        if prep_q:
                            prep_q.pop(0)()
                    # budget-weighted PV drain: a chain for qt costs qt+1
                    # matmuls; cap the PE time added per group
                    budget = pv_budget_last if h == HPC - 1 else pv_budget
                    while pv_q and budget > 0:
                        hh, qq = pv_q.pop(0)
                        emit_pv(hh, qq)
                        budget -= qq + 1
                # strips all done at end of head's groups
                while pv_next[h] < NT:
                    pv_q.append((h, pv_next[h]))
                    pv_next[h] += 1

            while prep_q:
                prep_q.pop(0)()
            while pv_q:
                hh, qq = pv_q.pop(0)
                emit_pv(hh, qq)

    _split_waits(nc)
    return nc


_NC = None


def kernel(query_states, key_states, value_states):
    global _NC
    qf = np.ascontiguousarray(
        np.asarray(query_states, dtype=np.float32).reshape(B * H, S, D))
    kf = np.ascontiguousarray(
        np.asarray(key_states, dtype=np.float32).reshape(B * H, S, D))
    vf = np.ascontiguousarray(
        np.asarray(value_states, dtype=np.float32).reshape(B * H, S, D))

    if _NC is None:
        _NC = build_nc2()

    in_maps = [
        {"q": qf[i * HPC:(i + 1) * HPC],
         "k": kf[i * HPC:(i + 1) * HPC],
         "v": vf[i * HPC:(i + 1) * HPC]}
        for i in range(NCORES)
    ]
    res = run_bass_kernel_spmd(_NC, in_maps, core_ids=list(range(NCORES)))
    out = np.concatenate([res.results[i]["o"] for i in range(NCORES)], axis=0)
    return out.reshape(B, H, S, D)

